# revision 17
# baseline (speedup 1.0000x reference)
"""Trainium2 Bass kernel for packed-sequence attention (nn_Attention).

Sharding (8 cores): core c handles sequence c//2 and head-group c%2
(8 of 16 heads).  Each core runs an identical SPMD program in four
phases, ordered so the tensor engine streams with minimal LDWEIGHTS
(stationary operands are reused across consecutive matmuls and the
redundant weight reloads are deleted from the BIR):
  P0) V projection for all 8 local heads (x chunks stationary, two
      512-wide weight streams per chunk) -> vv [t, head*d] bf16
  P1) Q/K projection (weights stationary, two token-panel streams
      each) + fused RMSNorm (ones-matmul cross-partition sumsq) +
      RoPE (pair-swap via SBUF-SBUF DMA in a deinterleaved d-basis
      baked into the host-permuted wqkv rows); qr/kr spilled to DRAM
  P2) attention per head: S = K'Q (two q-panels per stationary K
      chunk) -> 2-wide exp on ACT -> PV + ones-row-sum matmuls in
      PSUM; normalize with the row-sum reciprocal broadcast
  P3) wo: o chunks stationary, four 512-wide wo streams each
Host: pairs of cores holding the same sequence have complementary head
groups; their partial outputs are summed (row-parallel TP unshard).
"""

import json
import math
import numpy as np
import ml_dtypes
from contextlib import ExitStack

P = 128
HD = 128
BF = ml_dtypes.bfloat16


def _dedup_ldweights(nc, mybir):
    """Remove InstLdweights that reload the identical stationary operand.

    The tile scheduler emits one LDWEIGHTS per matmul; consecutive
    matmuls sharing lhsT reload the PE array needlessly (~46ns each on
    HW).  The final BIR marks every matmul ldweights=False with a
    separate InstLdweights, so dropping a redundant, sync-free reload
    leaves a supported load-once/matmul-repeatedly stream.
    """
    def sig(inst):
        j = json.loads(mybir.instruction_to_pretty_json_string(inst))
        j.pop("name", None)
        j.pop("sync_info", None)
        return json.dumps(j, sort_keys=True)

    removed = 0
    for f in nc.m.functions:
        for bb in f.blocks:
            last = None
            to_del = []
            for i, inst in enumerate(bb.instructions):
                if getattr(inst, "engine", None) != mybir.EngineType.PE:
                    continue
                if isinstance(inst, mybir.InstLdweights):
                    s = sig(inst)
                    si = inst.sync_info
                    empty = si is None or (
                        len(si.on_wait) == 0 and len(si.on_update) == 0)
                    if empty and s == last:
                        to_del.append(i)
                        removed += 1
                    else:
                        last = s
                elif isinstance(inst, (mybir.InstMatmult,
                                       mybir.InstEventSemaphore)):
                    pass
                else:
                    last = None
            for i in reversed(to_del):
                del bb.instructions[i]
    return removed


def _build_program(L, C, NP, DOUT, n_cores):
    """Build the SPMD per-core program.

    L: tokens per core (sequence length), C: model/contraction dim,
    NP: local head pairs (local heads = 2*NP), DOUT: wo output dim.
    """
    import concourse.bass as bass
    import concourse.mybir as mybir
    import concourse.tile as tile
    from concourse import bacc

    dt = mybir.dt
    AF = mybir.ActivationFunctionType
    OP = mybir.AluOpType

    NHL = 2 * NP           # local heads (8)
    TP = L // 512          # token/query panels (4)
    KC = L // P            # key chunks / token sub-tiles (16)
    CCH = C // P           # contraction chunks (16)
    JP = DOUT // 512       # wo output column panels (4)
    scale = 1.0 / math.sqrt(HD)
    EXPB = -math.log(16.0)  # exp(s*scale - ln16): keeps es in [~0, 20]
    EPS = 1e-5

    nc = bacc.Bacc("TRN2", target_bir_lowering=False, debug=False,
                   num_devices=n_cores)

    xT_d = nc.dram_tensor("xT", [C, L], dt.bfloat16, kind="ExternalInput").ap()
    wqkT_d = nc.dram_tensor("wqkT", [C, NP * 512], dt.bfloat16,
                            kind="ExternalInput").ap()
    wvT_d = nc.dram_tensor("wvT", [C, NHL * HD], dt.bfloat16,
                           kind="ExternalInput").ap()
    woT_d = nc.dram_tensor("woT", [NHL * HD, DOUT], dt.bfloat16,
                           kind="ExternalInput").ap()
    cosT_d = nc.dram_tensor("cosT", [P, L], dt.bfloat16, kind="ExternalInput").ap()
    sinT_d = nc.dram_tensor("sinT", [P, L], dt.bfloat16, kind="ExternalInput").ap()
    qnw_d = nc.dram_tensor("qnw", [P, 1], dt.float32, kind="ExternalInput").ap()
    knw_d = nc.dram_tensor("knw", [P, 1], dt.float32, kind="ExternalInput").ap()
    out_d = nc.dram_tensor("out", [L, DOUT], dt.float32, kind="ExternalOutput").ap()
    import os
    DBG = bool(os.environ.get("KDBG"))
    if DBG:
        dbg_vv = nc.dram_tensor("dbg_vv", [P, KC, NHL * HD], dt.bfloat16,
                                kind="ExternalOutput").ap()
        dbg_u = [nc.dram_tensor(f"dbg_u{u}", [P, L], dt.bfloat16,
                                kind="ExternalOutput").ap() for u in range(4)]
        dbg_o = nc.dram_tensor("dbg_o", [P, L], dt.bfloat16,
                               kind="ExternalOutput").ap()
        dbg_d = nc.dram_tensor("dbg_d", [2, 512], dt.float32,
                               kind="ExternalOutput").ap()
        dbg_po = nc.dram_tensor("dbg_po", [P, 2, 512], dt.float32,
                                kind="ExternalOutput").ap()

    with tile.TileContext(nc) as tc:
        with ExitStack() as ctx:
            const = ctx.enter_context(tc.tile_pool(name="const", bufs=1))
            vvp = ctx.enter_context(tc.tile_pool(name="vvp", bufs=1))
            dpool = ctx.enter_context(tc.tile_pool(name="dpool", bufs=4,
                                                   space="DRAM"))
            spill = ctx.enter_context(tc.tile_pool(name="spill", bufs=16,
                                                   space="DRAM"))

            ones_sb = const.tile([P, 1], dt.bfloat16, tag="ones", bufs=1)
            nc.vector.memset(ones_sb[:], 1.0)
            eps_sb = const.tile([P, 1], dt.float32, tag="eps", bufs=1)
            nc.vector.memset(eps_sb[:], EPS)
            expb_sb = const.tile([P, 1], dt.float32, tag="expb", bufs=1)
            nc.vector.memset(expb_sb[:], EXPB)
            qnw_sb = const.tile([P, 1], dt.float32, tag="qnw", bufs=1)
            nc.sync.dma_start(qnw_sb[:], qnw_d[:])
            knw_sb = const.tile([P, 1], dt.float32, tag="knw", bufs=1)
            nc.sync.dma_start(knw_sb[:], knw_d[:])

            vv_sb = vvp.tile([P, KC, NHL * HD], dt.bfloat16, tag="vv", bufs=1)
            qkp = ctx.enter_context(tc.tile_pool(name="qkp", bufs=4))

            qk_dram = [[spill.tile([P, L], dt.bfloat16, tag=f"qkd{p}_{u}",
                                   bufs=1, name=f"qkd{p}_{u}")
                        for u in range(4)] for p in range(NP)]

            with ExitStack() as pxs:   # x resident for P0 + P1
                xp = pxs.enter_context(tc.tile_pool(name="xp", bufs=1))
                x_sb = xp.tile([P, CCH, L], dt.bfloat16, tag="x", bufs=1)
                xs = xT_d.rearrange("(cc q) w -> q cc w", q=P)

                # =============== P0: V projection (all heads) ===========
                with ExitStack() as p0:
                    wvp = p0.enter_context(tc.tile_pool(name="wvp", bufs=1))
                    p0ps = p0.enter_context(tc.tile_pool(name="p0ps", bufs=2,
                                                         space="PSUM"))
                    wv_sb = wvp.tile([P, CCH, NHL * HD], dt.bfloat16, tag="wv",
                                     bufs=1)
                    wvs = wvT_d.rearrange("(cc q) w -> q cc w", q=P)
                    for sp in range(4):
                        c0, c1 = sp * CCH // 4, (sp + 1) * CCH // 4
                        nc.sync.dma_start(wv_sb[:, c0:c1, :], wvs[:, c0:c1, :])
                    for hw in range(2):
                        hsl = bass.ts(hw, L // 2)
                        eng = nc.sync if hw == 0 else nc.scalar
                        for cc in range(CCH):
                            eng.dma_start(x_sb[:, cc, hsl], xs[:, cc, hsl])

                    pend0 = None
                    for ts in range(KC):
                        pva = p0ps.tile([P, 512], dt.float32, tag="pva", bufs=2)
                        pvb = p0ps.tile([P, 512], dt.float32, tag="pvb", bufs=2)
                        tsl = bass.ts(ts, P)
                        for cc in range(CCH):
                            st = x_sb[:, cc, tsl]
                            nc.tensor.matmul(pva[:], st, wv_sb[:, cc, 0:512],
                                             start=(cc == 0),
                                             stop=(cc == CCH - 1))
                            nc.tensor.matmul(pvb[:], st, wv_sb[:, cc, 512:1024],
                                             start=(cc == 0),
                                             stop=(cc == CCH - 1))
                        nc.vector.tensor_copy(vv_sb[:, ts, 0:512], pva[:])
                        nc.vector.tensor_copy(vv_sb[:, ts, 512:1024], pvb[:])

                if DBG:
                    nc.sync.dma_start(dbg_vv[:], vv_sb[:])
                # ============ P1: Q/K projection + RMSNorm + RoPE ========
                with ExitStack() as p1:
                    cosp = p1.enter_context(tc.tile_pool(name="cosp", bufs=1))
                    wqp = p1.enter_context(tc.tile_pool(name="wqp", bufs=2))
                    p1ps = p1.enter_context(tc.tile_pool(name="p1ps", bufs=2,
                                                         space="PSUM"))
                    work = p1.enter_context(tc.tile_pool(name="w1", bufs=3))
                    cos_sb = cosp.tile([P, L], dt.bfloat16, tag="cos", bufs=1)
                    nc.sync.dma_start(cos_sb[:], cosT_d[:])
                    sin_sb = cosp.tile([P, L], dt.bfloat16, tag="sin", bufs=1)
                    nc.sync.dma_start(sin_sb[:], sinT_d[:])

                    def emit_tail(st):
                        """ssq matmuls + rmsnorm/rope for a finished unit.

                        Called one unit later so the ssq matmuls never stall
                        the PE waiting on the DVE square chain.
                        """
                        (dest, wv_i, wnorm, ssq, q2t) = st
                        for tp2 in range(2):
                            nc.tensor.matmul(ssq[32 * tp2:32 * tp2 + 1, :],
                                             ones_sb[:], q2t[tp2][1][:],
                                             start=True, stop=True)
                        for tp2 in range(2):
                            tsl = bass.ts(wv_i * 2 + tp2, 512)
                            qraw = q2t[tp2][0]
                            rms = work.tile([1, 512], dt.float32, tag="rms",
                                            bufs=2)
                            nc.scalar.activation(
                                rms[:], ssq[32 * tp2:32 * tp2 + 1, :],
                                AF.Sqrt, bias=eps_sb[0:1, :], scale=1.0 / HD)
                            rs = work.tile([1, 512], dt.float32, tag="rs",
                                           bufs=2)
                            nc.vector.reciprocal_approx_fast(rs[:], rms[:])
                            rsd = dpool.tile([1, 512], dt.float32,
                                             tag="rsd", bufs=4)
                            nc.gpsimd.dma_start(rsd[:], rs[:])
                            rsb = work.tile([P, 512], dt.float32, tag="rsb",
                                            bufs=2)
                            nc.gpsimd.dma_start(rsb[:],
                                                rsd[:].to_broadcast((P, 512)))
                            qs = work.tile([P, 512], dt.bfloat16, tag="qs",
                                           bufs=3)
                            nc.vector.scalar_tensor_tensor(
                                qs[:], qraw[:], wnorm[:], rsb[:],
                                op0=OP.mult, op1=OP.mult)
                            qsw = work.tile([P, 512], dt.bfloat16,
                                            tag="qsw", bufs=3)
                            nc.gpsimd.dma_start(qsw[0:64, :], qs[64:128, :])
                            nc.gpsimd.dma_start(qsw[64:128, :], qs[0:64, :])
                            t1 = work.tile([P, 512], dt.bfloat16, tag="t1",
                                           bufs=2)
                            nc.vector.tensor_mul(t1[:], qs[:], cos_sb[:, tsl])
                            t2 = work.tile([P, 512], dt.bfloat16, tag="t2",
                                           bufs=2)
                            nc.vector.tensor_mul(t2[:], qsw[:], sin_sb[:, tsl])
                            nc.vector.tensor_add(dest[:, tsl], t1[:], t2[:])

                    pending = None
                    spill_pend = None
                    uq_last = None
                    wqk_tiles = {}

                    def load_wqk(p):
                        wqk_sb = wqp.tile([P, CCH, 512], dt.bfloat16, tag="wqk",
                                          bufs=2, name=f"wqk{p}")
                        wqs = wqkT_d[:, p * 512:(p + 1) * 512].rearrange(
                            "(cc q) w -> q cc w", q=P)
                        for sp in range(4):
                            c0, c1 = sp * CCH // 4, (sp + 1) * CCH // 4
                            nc.sync.dma_start(wqk_sb[:, c0:c1, :],
                                              wqs[:, c0:c1, :])
                        wqk_tiles[p] = wqk_sb

                    load_wqk(0)
                    for p in range(NP):
                        if p + 1 < NP:
                            load_wqk(p + 1)     # prefetch next pair early
                        wqk_sb = wqk_tiles.pop(p)

                        uq = [qkp.tile([P, L], dt.bfloat16, tag=f"u{u}",
                                       bufs=1, name=f"u{p}_{u}")
                              for u in range(4)]
                        for wv_ in range(2):      # two token-panel waves
                            for u in range(4):    # q0 q1 k0 k1
                                pq = p1ps.tile([P, 2, 512], dt.float32,
                                               tag="pq", bufs=3)
                                for cc in range(CCH):
                                    wst = wqk_sb[:, cc, u * P:(u + 1) * P]
                                    for tp2 in range(2):
                                        nc.tensor.matmul(
                                            pq[:, tp2, :], wst,
                                            x_sb[:, cc,
                                                 bass.ts(wv_ * 2 + tp2, 512)],
                                            start=(cc == 0),
                                            stop=(cc == CCH - 1))
                                # DVE square chain starts right away ...
                                ssq = p1ps.tile([P, 512], dt.float32,
                                                tag="ssq", bufs=1)
                                q2t = []
                                for tp2 in range(2):
                                    qraw = work.tile([P, 512], dt.float32,
                                                     tag="qraw", bufs=4)
                                    nc.vector.tensor_copy(qraw[:], pq[:, tp2, :])
                                    q2 = work.tile([P, 512], dt.bfloat16,
                                                   tag="q2", bufs=4)
                                    nc.vector.tensor_mul(q2[:], qraw[:], qraw[:])
                                    q2t.append((qraw, q2))
                                # ... but its PE ssq matmuls wait one unit
                                if pending is not None:
                                    emit_tail(pending)
                                if spill_pend is not None:
                                    nc.sync.dma_start(*spill_pend)
                                    spill_pend = None
                                wnorm = qnw_sb if u < 2 else knw_sb
                                pending = (uq[u], wv_, wnorm, ssq, q2t)
                            if wv_ == 1:
                                for u in range(4):
                                    if u < 3:
                                        nc.sync.dma_start(qk_dram[p][u][:],
                                                          uq[u][:])
                                    else:
                                        spill_pend = (qk_dram[p][u][:], uq[u][:])
                    emit_tail(pending)
                    nc.sync.dma_start(*spill_pend)

            if DBG:
                for u in range(4):
                    nc.sync.dma_start(dbg_u[u][:], qk_dram[0][u][:])
            # ================= P2: attention per head ==================
            op_ = ctx.enter_context(tc.tile_pool(name="op", bufs=1))
            o_tiles = [op_.tile([P, L], dt.bfloat16, tag=f"o{h}", bufs=1,
                                name=f"o{h}") for h in range(NHL)]
            with ExitStack() as p2:
                qrp = p2.enter_context(tc.tile_pool(name="qrp", bufs=2))
                esp = p2.enter_context(tc.tile_pool(name="esp", bufs=4))
                work = p2.enter_context(tc.tile_pool(name="w2", bufs=3))
                p2ps = p2.enter_context(tc.tile_pool(name="p2ps", bufs=2,
                                                     space="PSUM"))
                dsum_pend = []
                for p in range(NP):
                    qk_sb = [qrp.tile([P, L], dt.bfloat16, tag=f"qk{u}",
                                      bufs=2, name=f"qka{p}_{u}")
                             for u in range(4)]
                    for u in range(4):
                        nc.sync.dma_start(qk_sb[u][:], qk_dram[p][u][:])
                    for l in range(2):
                        qr, kr = qk_sb[l], qk_sb[2 + l]
                        h = 2 * p + l
                        hsl = slice(h * HD, (h + 1) * HD)
                        for wv_ in range(2):
                            psl = [bass.ts(wv_ * 2 + i, 512) for i in range(2)]
                            po = p2ps.tile([P, 2, 512], dt.float32, tag="po",
                                           bufs=1)
                            es_t = [None] * KC
                            pend = {}
                            drain_dsum = dsum_pend and dsum_pend.pop(0)

                            def push(cur, lvl):
                                # binary-counter pairwise tree over es chunks
                                while lvl in pend:
                                    prev = pend.pop(lvl)
                                    nt = work.tile([P, 2, 512], dt.bfloat16,
                                                   tag=f"sm{min(lvl, 3)}",
                                                   bufs=2, name=f"sm{lvl}")
                                    nc.vector.tensor_add(nt[:], prev[:], cur[:])
                                    cur = nt
                                    lvl += 1
                                pend[lvl] = cur

                            def emit_pv(kc):
                                vst = vv_sb[:, kc, hsl]
                                for i in range(2):
                                    nc.tensor.matmul(po[:, i, :], vst,
                                                     es_t[kc][:, i, :],
                                                     start=(kc == 0),
                                                     stop=(kc == KC - 1))

                            for kc in range(KC):
                                s2 = p2ps.tile([P, 2, 512], dt.float32,
                                               tag="s2", bufs=2)
                                kst = kr[:, kc * P:(kc + 1) * P]
                                for i in range(2):
                                    nc.tensor.matmul(s2[:, i, :], kst,
                                                     qr[:, psl[i]],
                                                     start=True, stop=True)
                                es = esp.tile([P, 2, 512], dt.bfloat16,
                                              tag="es", bufs=4)
                                nc.scalar.activation(es[:], s2[:], AF.Exp,
                                                     bias=expb_sb[:],
                                                     scale=scale)
                                es_t[kc] = es
                                if kc == 2 and drain_dsum:
                                    drain_dsum()
                                if kc < KC - 1:
                                    push(es, 0)
                                # PV lags S by 2 chunks so the PE never
                                # blocks on the exp of the chunk it needs
                                if kc >= 2:
                                    emit_pv(kc - 2)
                            emit_pv(KC - 2)
                            emit_pv(KC - 1)
                            po_sb = work.tile([P, 2, 512], dt.float32,
                                              tag="posb", bufs=2)
                            nc.vector.tensor_copy(po_sb[:], po[:])
                            if DBG and h == 0 and wv_ == 0:
                                nc.sync.dma_start(dbg_po[:], po_sb[:])
                            # last leaf + carry chain after the po drain so
                            # the DVE frees po before the next wave's PV
                            push(es_t[KC - 1], 0)
                            es_sum = pend.pop(4)
                            assert not pend

                            def emit_dsum(h=h, wv_=wv_, psl=psl, po_sb=po_sb,
                                          es_sum=es_sum):
                                prs = p2ps.tile([1, 1024], dt.float32,
                                                tag="prs", bufs=1)
                                nc.tensor.matmul(prs[0:1, 0:512], ones_sb[:],
                                                 es_sum[:, 0, :],
                                                 start=True, stop=True)
                                nc.tensor.matmul(prs[0:1, 512:1024], ones_sb[:],
                                                 es_sum[:, 1, :],
                                                 start=True, stop=True)
                                for i in range(2):
                                    rr = work.tile([1, 512], dt.float32,
                                                   tag="rr", bufs=2)
                                    nc.vector.reciprocal_approx_fast(
                                        rr[:], prs[0:1, 512 * i:512 * (i + 1)])
                                    if DBG and h == 0 and wv_ == 0:
                                        nc.sync.dma_start(dbg_d[i:i + 1, :], rr[:])
                                    rrd = dpool.tile([1, 512], dt.float32,
                                                     tag="rrd", bufs=4)
                                    nc.gpsimd.dma_start(rrd[:], rr[:])
                                    rrb = work.tile([P, 512], dt.float32,
                                                    tag="rrb", bufs=2)
                                    nc.gpsimd.dma_start(
                                        rrb[:], rrd[:].to_broadcast((P, 512)))
                                    nc.vector.tensor_mul(o_tiles[h][:, psl[i]],
                                                         po_sb[:, i, :], rrb[:])
                            dsum_pend.append(emit_dsum)

                for fn in dsum_pend:
                    fn()
            if DBG:
                nc.sync.dma_start(dbg_o[:], o_tiles[0][:])
            # ======================= P3: wo ============================
            with ExitStack() as p3:
                wop = p3.enter_context(tc.tile_pool(name="wop", bufs=1))
                work = p3.enter_context(tc.tile_pool(name="w3", bufs=3))
                p3ps = p3.enter_context(tc.tile_pool(name="p3ps", bufs=2,
                                                     space="PSUM"))
                wo_sb = wop.tile([P, NHL, DOUT], dt.bfloat16, tag="wo", bufs=1)
                wos = woT_d.rearrange("(h q) j -> q h j", q=P)
                for h in range(NHL):
                    nc.sync.dma_start(wo_sb[:, h, :], wos[:, h, :])
                for tt in range(KC):
                    pw = p3ps.tile([P, JP, 512], dt.float32, tag="pw", bufs=2)
                    ttsl = bass.ts(tt, P)
                    for h in range(NHL):
                        ost = o_tiles[h][:, ttsl]
                        for jp in range(JP):
                            nc.tensor.matmul(pw[:, jp, :], ost,
                                             wo_sb[:, h, bass.ts(jp, 512)],
                                             start=(h == 0),
                                             stop=(h == NHL - 1))
                    for jp in range(JP):
                        osb = work.tile([P, 512], dt.float32, tag="outsb",
                                        bufs=3)
                        nc.vector.tensor_copy(osb[:], pw[:, jp, :])
                        nc.sync.dma_start(out_d[ttsl, bass.ts(jp, 512)], osb[:])

    import concourse.mybir as mybir_
    n_rm = _dedup_ldweights(nc, mybir_)
    print(f"[kernel] dedup removed {n_rm} redundant ldweights")
    nc.compile()
    return nc


def _host_prepare(x, rope_cos, rope_sin, wqkv, wo, q_norm_w, k_norm_w,
                  L, C, NP, DOUT, n_cores):
    """Build per-core input dicts."""
    NH_TOT = wqkv.shape[0] // 3 // HD
    NHL = 2 * NP
    perm = np.concatenate([np.arange(0, HD, 2), np.arange(1, HD, 2)])  # deinterleave

    qn_p = np.ascontiguousarray(q_norm_w[perm].reshape(HD, 1)).astype(np.float32)
    kn_p = np.ascontiguousarray(k_norm_w[perm].reshape(HD, 1)).astype(np.float32)

    wq = wqkv[0 * NH_TOT * HD:1 * NH_TOT * HD].reshape(NH_TOT, HD, C)
    wk = wqkv[1 * NH_TOT * HD:2 * NH_TOT * HD].reshape(NH_TOT, HD, C)
    wv = wqkv[2 * NH_TOT * HD:3 * NH_TOT * HD].reshape(NH_TOT, HD, C)

    in_maps = []
    for c in range(n_cores):
        b = c // 2
        hg = c % 2
        heads = list(range(hg * NHL, hg * NHL + NHL))
        xb = x[b * L:(b + 1) * L]                       # [L, C]
        xT = np.ascontiguousarray(xb.T).astype(BF)      # [C, L]

        qk_blocks = []
        for pidx in range(NP):
            h0, h1 = heads[2 * pidx], heads[2 * pidx + 1]
            qk_blocks += [wq[h0][perm], wq[h1][perm],
                          wk[h0][perm], wk[h1][perm]]
        wqkT = np.ascontiguousarray(
            np.concatenate(qk_blocks, axis=0).T).astype(BF)   # [C, NP*512]
        wvT = np.ascontiguousarray(
            np.concatenate([wv[h] for h in heads], axis=0).T).astype(BF)

        woT_rows = wo[:, heads[0] * HD:(heads[-1] + 1) * HD].T  # [NHL*HD, DOUT]
        woT = np.ascontiguousarray(woT_rows).astype(BF)

        cosb = rope_cos[b * L:(b + 1) * L].T            # [64, L]
        sinb = rope_sin[b * L:(b + 1) * L].T
        cosT = np.ascontiguousarray(np.concatenate([cosb, cosb], 0)).astype(BF)
        sinT = np.ascontiguousarray(np.concatenate([-sinb, sinb], 0)).astype(BF)

        in_maps.append({
            "xT": xT, "wqkT": wqkT, "wvT": wvT, "woT": woT,
            "cosT": cosT, "sinT": sinT, "qnw": qn_p, "knw": kn_p,
        })
    return in_maps


def _reference_numpy(x, rope_cos, rope_sin, cu, max_length,
                     wqkv, wo, q_norm_w, k_norm_w):
    """Pure-numpy fallback (exact reference math) for non-uniform cu."""
    T, dim = x.shape
    nh = dim // HD
    qkv = (x @ wqkv.T).reshape(T, 3, nh, HD)
    q, k, v = qkv[:, 0], qkv[:, 1], qkv[:, 2]

    def rmsnorm(t, w):
        return t / np.sqrt((t * t).mean(-1, keepdims=True) + 1e-5) * w

    def rope(t):
        tr = t.reshape(t.shape[:-1] + (HD // 2, 2))
        e, o = tr[..., 0], tr[..., 1]
        cc = rope_cos[:, None, :]
        ss = rope_sin[:, None, :]
        return np.stack([e * cc - o * ss, e * ss + o * cc], -1).reshape(t.shape)

    q = rope(rmsnorm(q, q_norm_w))
    k = rope(rmsnorm(k, k_norm_w))
    o = np.zeros((T, nh, HD), np.float32)
    nb = len(cu) - 1
    for i in range(nb):
        s, e_ = int(cu[i]), int(cu[i + 1])
        if e_ <= s:
            continue
        qs_, ks_, vs_ = q[s:e_], k[s:e_], v[s:e_]
        sc = np.einsum("lhd,mhd->hlm", qs_, ks_) / math.sqrt(HD)
        sc = sc - sc.max(-1, keepdims=True)
        a = np.exp(sc)
        a /= a.sum(-1, keepdims=True)
        o[s:e_] = np.einsum("hlm,mhd->lhd", a, vs_)
    return (o.reshape(T, dim) @ wo.T).astype(np.float32)


def kernel(x, rope_cos, rope_sin, cu, max_length, wqkv, wo, q_norm_w, k_norm_w):
    x = np.asarray(x, np.float32)
    rope_cos = np.asarray(rope_cos, np.float32)
    rope_sin = np.asarray(rope_sin, np.float32)
    cu = np.asarray(cu)
    wqkv = np.asarray(wqkv, np.float32)
    wo = np.asarray(wo, np.float32)
    q_norm_w = np.asarray(q_norm_w, np.float32)
    k_norm_w = np.asarray(k_norm_w, np.float32)

    T, C = x.shape
    N_CORES = 8
    L = T // 4
    expect_cu = np.arange(5) * L
    if (len(cu) != 5 or not np.array_equal(np.asarray(cu).ravel(), expect_cu)
            or T % 4 != 0 or L % 512 != 0 or C % P != 0):
        return _reference_numpy(x, rope_cos, rope_sin, cu, max_length,
                                wqkv, wo, q_norm_w, k_norm_w)

    NP = (C // HD) // 2 // 2          # local head pairs = NH/2/2
    DOUT = wo.shape[0]

    from concourse.bass_utils import run_bass_kernel_spmd

    nc = _build_program(L, C, NP, DOUT, N_CORES)
    in_maps = _host_prepare(x, rope_cos, rope_sin, wqkv, wo, q_norm_w, k_norm_w,
                            L, C, NP, DOUT, N_CORES)
    res = run_bass_kernel_spmd(nc, in_maps, list(range(N_CORES)))

    out = np.empty((T, DOUT), np.float32)
    for b in range(4):
        out[b * L:(b + 1) * L] = (res.results[2 * b]["out"]
                                  + res.results[2 * b + 1]["out"])
    return out
